# revision 36
# baseline (speedup 1.0000x reference)
"""Trainium2 Bass kernel for ConvexContractionAttention (v2: bf16 + fused ops).

Math (derived from the reference; see kernel_baseline.py for the derivation):
  per channel d with beta == 0:
      p    = (xq - muq) * (xk - muk)
      s_j  = sigmoid(cj * p),  cj = gamma*Aq_j*Ak_j
      tt   = (sum_j Av_j s_j) / (sum_j s_j + eps_w)
      out0 = (xv - muv) * tt
      out  = (out0 - mean(out0)) * g_out/std(out0) + b_out
  A*_j = u_j*g*rsqrt(var*u_j^2 + eps_norm), u = softplus(w) @ qr(a).Q (host).

Engine plan per core (128 channels on partitions, BT=8192 on free axis):
  - x shipped as bf16, twice: [ch, f] for the main loop, and block-transposed
    [f%128, block, 129] = [x_block^T | ones] so one PE matmul chain per chunk
    accumulates M = [gram | colsum] in PSUM; diag(gram) = sum x^2 (extracted
    with one STT+accum against the identity), last col = sum x. All per-chunk
    mean/var stats cost only TensorE time, which is otherwise idle.
  - p via one fused custom-DVE op CENTER2_MUL: (xq-muq)*(xk-muk) -> bf16.
  - 3 sigmoids on ScalarE (scale=cj per partition) -> bf16.
  - den = (s0+s1)+s2 on DVE (bf16 TT 2x); num via TensorE diag(Av_j) matmuls
    accumulated in PSUM (custom-DVE ops cannot read two PSUM operands; walrus
    rejects them, so den stays in SBUF and num rides in1 which may be PSUM).
  - tt via one fused custom-DVE op RECIP_MUL: num * recip1NR(den+eps) -> bf16
    (one-Newton bit-trick reciprocal, minimax-tuned, 0.17% max error).
  - out0 via STT (xv-muv)*tt with accum_out -> osum; osq via ScalarE Square.
  - final affine via DVE tensor_scalar; output stored bf16 (host upcasts),
    output DMA on the GpSimd software-DGE queue so stores never block loads.
  - All tile pools are created once in build_program and shared across reps
    (bufs=2) so consecutive bodies pipeline; per-body pools would alias SBUF
    and serialize body boundaries.
"""

import sys

if "/opt/trn_rl_repo" not in sys.path:
    sys.path.insert(0, "/opt/trn_rl_repo")

import numpy as np

import concourse.bacc as bacc
import concourse.tile as tile
from concourse import mybir
from concourse import bass_utils

B, T, D = 4, 2048, 1024
BT = B * T
N_CORES = 8
DL = D // N_CORES
GAMMA = 5.0
EPS_NORM = 1e-5
EPS_W = 1e-8

F32 = mybir.dt.float32
BF16 = mybir.dt.bfloat16
Act = mybir.ActivationFunctionType
Alu = mybir.AluOpType

# minimax constants for the one-NR reciprocal y = NOT(x)*(RA - RB*x*NOT(x)),
# |y*x-1| <= 1.73e-3 over x in [1e-30, 3.1]
RA = -0.47140361
RB = 0.05545923

F_MAIN = 2048          # main-loop tile
F_PE = 1024            # PSUM den/num tile (2 banks each)


# --- custom DVE op registration -------------------------------------------
def _register_custom_ops():
    import concourse.dve_ops as dve_ops_mod
    from concourse.dve_ops import DveOp, _SUB_OPCODE_FOR_NAME, _CUSTOM_DVE_ROW_BASE
    from concourse.dve_spec import Spec, Src0, Src1, C0, C1, C2, Bin, AluOp, lower
    from concourse.dve_uop import DveOpSpec

    if "CENTER2_MUL_ANT" in _SUB_OPCODE_FOR_NAME:
        by_name = {op.name: op for op in dve_ops_mod.OPS}
        return by_name["CENTER2_MUL_ANT"], by_name["RECIP_MUL_ANT"]

    spec_center2 = Spec(
        body=(Src0 - C0) * (Src1 - C1),
        reference=lambda in0, in1, s0, s1, imm2: (
            (in0.astype(np.float32) - s0) * (in1.astype(np.float32) - s1)
        ).astype(np.float32),
    )

    _x = Src0 + C2
    _nx = Bin(AluOp.BITWISE_NOT, _x, _x)
    _y = _nx * (C0 - C1 * (_x * _nx))

    def _ref_recip_mul(in0, in1, s0, s1, imm2):
        x = in0.astype(np.float32) + np.float32(imm2)
        nx = (~x.view(np.int32)).view(np.float32)
        y = nx * (np.float32(s0) - np.float32(s1) * (x * nx))
        return (in1.astype(np.float32) * y).astype(np.float32)

    spec_recip_mul = Spec(body=Src1 * _y, reference=_ref_recip_mul)

    def reg(name, spec):
        row = _CUSTOM_DVE_ROW_BASE + len(dve_ops_mod.OPS)
        assert row < 0x20
        _SUB_OPCODE_FOR_NAME[name] = row
        sha = DveOpSpec(name=name, opcode=row, uops=lower(spec, ver="v3"),
                        rd1_en=True).sha("v3")
        op = DveOp(name, spec, subdim=False, uops_sha={"v3": sha})
        dve_ops_mod.OPS.append(op)
        dve_ops_mod.CUSTOM_DVE_SPECS[name] = spec
        return op

    return (reg("CENTER2_MUL_ANT", spec_center2),
            reg("RECIP_MUL_ANT", spec_recip_mul))


OP_CENTER2, OP_RECIPMUL = _register_custom_ops()


def _emit_rsqrt(nc, pool, v, n, tag):
    """out = 1/sqrt(v) on a tiny [DL, n] fp32 tile (bit-trick + 3 Newton)."""
    U32 = mybir.dt.uint32
    bitsf = pool.tile([DL, n], F32, name=f"rsq_b_{tag}", tag=f"rsq_b_{tag}")
    nc.vector.tensor_copy(bitsf, v.bitcast(U32))
    nc.vector.tensor_scalar(
        out=bitsf, in0=bitsf, scalar1=-0.5, scalar2=1597463007.0,
        op0=Alu.mult, op1=Alu.add,
    )
    yu = pool.tile([DL, n], U32, name=f"rsq_y_{tag}", tag=f"rsq_y_{tag}")
    nc.vector.tensor_copy(yu, bitsf)
    y = yu.bitcast(F32)
    t = pool.tile([DL, n], F32, name=f"rsq_t_{tag}", tag=f"rsq_t_{tag}")
    for _ in range(2):
        nc.vector.tensor_mul(t, y, y)
        nc.vector.tensor_mul(t, t, v)
        nc.vector.tensor_scalar(
            out=t, in0=t, scalar1=-0.5, scalar2=1.5, op0=Alu.mult, op1=Alu.add,
        )
        nc.vector.tensor_mul(y, y, t)
    return y


def _emit_body(nc, tc, dram, pools):
    if True:
        resident, out0pool, consts, work, psum = pools

        chunks = ("q", "k", "v")

        # tiny parameter DMAs first
        usq = consts.tile([DL, 9], F32, name="usq", tag="usq")
        ug_all = consts.tile([DL, 9], F32, name="ug_all", tag="ug_all")
        nc.sync.dma_start(out=usq, in_=dram["usq_all"])
        nc.sync.dma_start(out=ug_all, in_=dram["ug_all"])
        g_out_sb = consts.tile([DL, 1], F32, name="g_out", tag="g_out")
        b_out_sb = consts.tile([DL, 1], F32, name="b_out", tag="b_out")
        nc.sync.dma_start(out=g_out_sb, in_=dram["g_out"])
        nc.sync.dma_start(out=b_out_sb, in_=dram["b_out"])
        ident_sb = consts.tile([DL, DL], BF16, name="ident", tag="ident")
        nc.sync.dma_start(out=ident_sb, in_=dram["ident"])

        # warm the sigmoid table set during input DMA
        warm = consts.tile([DL, 1], F32, name="warm", tag="warm")
        nc.vector.memset(warm, 0.0)
        nc.scalar.activation(warm, warm, Act.Sigmoid)
        epst = consts.tile([DL, 1], F32, name="epst", tag="epst")
        nc.vector.memset(epst, EPS_W)

        # ---- phase 1: x loads (bf16) + per-channel stats via PE gram -------
        # Host ships each chunk twice: x [ch, f] for the main loop, and
        # xTr [f%128, block, 129] = [x_block^T | ones] so that one PE matmul
        # chain per chunk accumulates M = [gram | colsum] in PSUM:
        # diag(gram) = sum(x^2), last col = sum(x).
        NQ = 4
        FQ = BT // NQ
        NB = BT // DL          # 64 transpose blocks per chunk
        NBQ = NB // NQ
        x_sb, mv, gram = {}, {}, {}
        gram_all = psum.tile([DL, 3, DL + 1], F32, name="gram_all",
                             tag="gram_all")
        for ci, p in enumerate(chunks):
            x_sb[p] = resident.tile([DL, BT], BF16, name=f"x_{p}", tag=f"x_{p}")
            for qi in range(NQ):
                sl = slice(qi * FQ, (qi + 1) * FQ)
                eng = nc.sync if (qi % 2 == 0) else nc.scalar
                eng.dma_start(out=x_sb[p][:, sl], in_=dram["x" + p][:, sl])
            gram[p] = gram_all[:, ci, :]
            NE = 8
            NBE = NB // NE
            for qi in range(NE):
                xtr = work.tile([DL, NBE, DL + 1], BF16, name=f"xtr{qi%2}",
                                tag=f"xtr{qi%2}")
                eng = nc.scalar if (qi % 2 == 0) else nc.sync
                eng.dma_start(
                    out=xtr,
                    in_=dram["xt" + p][:, qi * NBE:(qi + 1) * NBE, :])
                for b in range(NBE):
                    gi = qi * NBE + b
                    nc.tensor.matmul(out=gram[p],
                                     lhsT=xtr[:, b, 0:DL], rhs=xtr[:, b, :],
                                     start=(gi == 0), stop=(gi == NB - 1))

        for ci, p in enumerate(chunks):
            mv[p] = consts.tile([DL, 2], F32, name=f"mv_{p}", tag=f"mv_{p}")
            # mu = colsum/BT
            nc.vector.tensor_scalar_mul(out=mv[p][:, 0:1],
                                        in0=gram[p][:, DL:DL + 1],
                                        scalar1=1.0 / BT)
            # sumsq = diag(gram) via one STT+accum: (gram*1)*ident summed
            mdi = work.tile([DL, DL], F32, name="mdi", tag="mdi")
            ssq = consts.tile([DL, 1], F32, name=f"ssq_{p}", tag=f"ssq_{p}")
            nc.vector.scalar_tensor_tensor(
                out=mdi, in0=gram[p][:, 0:DL], scalar=1.0, in1=ident_sb,
                op0=Alu.mult, op1=Alu.mult, accum_out=ssq)
            musq = consts.tile([DL, 1], F32, name=f"musq_{p}", tag=f"musq_{p}")
            nc.vector.tensor_mul(musq, mv[p][:, 0:1], mv[p][:, 0:1])
            nc.vector.scalar_tensor_tensor(
                out=mv[p][:, 1:2], in0=ssq, scalar=1.0 / BT,
                in1=musq, op0=Alu.mult, op1=Alu.subtract)

        # ---- constants: A = ug*rsqrt(var*u^2+eps); host folded gamma into ug_q
        vterm = consts.tile([DL, 9], F32, name="vterm", tag="vterm")
        for ci, p in enumerate(chunks):
            nc.vector.tensor_scalar(
                out=vterm[:, 3 * ci:3 * ci + 3], in0=usq[:, 3 * ci:3 * ci + 3],
                scalar1=mv[p][:, 1:2], scalar2=EPS_NORM,
                op0=Alu.mult, op1=Alu.add,
            )
        inv = _emit_rsqrt(nc, consts, vterm, 9, "A")
        A_all = consts.tile([DL, 9], F32, name="A_all", tag="A_all")
        nc.vector.tensor_mul(A_all, ug_all, inv)
        cmat = consts.tile([DL, 3], F32, name="cmat", tag="cmat")
        nc.vector.tensor_mul(cmat, A_all[:, 0:3], A_all[:, 3:6])
        dg = []
        for j in range(3):
            d = consts.tile([DL, DL], BF16, name=f"dg{j}", tag=f"dg{j}")
            nc.vector.tensor_scalar_mul(out=d, in0=ident_sb,
                                        scalar1=A_all[:, 6 + j:7 + j])
            dg.append(d)

        muq = mv["q"][:, 0:1]
        muk = mv["k"][:, 0:1]
        muv = mv["v"][:, 0:1]

        # ---- phase 2: main loop -------------------------------------------
        NH = BT // F_MAIN
        out0 = out0pool.tile([DL, BT], BF16, name="out0", tag="out0")
        osum = consts.tile([DL, NH], F32, name="osum", tag="osum")
        osq = consts.tile([DL, NH], F32, name="osq", tag="osq")

        for h in range(NH):
            sl = slice(h * F_MAIN, (h + 1) * F_MAIN)
            if h % 2 == 0:
                psl2 = slice(h * F_MAIN, (h + 2) * F_MAIN)
                p_t2 = work.tile([DL, 2 * F_MAIN], BF16, name="p_t", tag="p_t")
                nc.vector._custom_dve(OP_CENTER2, out=p_t2,
                                      in0=x_sb["q"][:, psl2],
                                      in1=x_sb["k"][:, psl2],
                                      s0=muq, s1=muk)
            p_t = p_t2[:, (h % 2) * F_MAIN:(h % 2 + 1) * F_MAIN]
            s_t = []
            for j in range(3):
                s = work.tile([DL, F_MAIN], BF16, name=f"s{j}", tag=f"s{j}")
                nc.scalar.activation(s, p_t, Act.Sigmoid, scale=cmat[:, j:j + 1])
                s_t.append(s)
            # den = (s0+s1)+s2 on DVE (bf16 TT, 2x)
            d01 = work.tile([DL, F_MAIN], BF16, name="d01", tag="p_t")
            nc.vector.tensor_add(d01, s_t[0], s_t[1])
            den = work.tile([DL, F_MAIN], BF16, name="den", tag="den")
            nc.vector.tensor_add(den, d01, s_t[2])
            tt = work.tile([DL, F_MAIN], BF16, name="tt", tag="tt")
            for b in range(F_MAIN // F_PE):
                pb = (h * (F_MAIN // F_PE) + b) % 2
                pnum = psum.tile([DL, F_PE], F32, name=f"pn{pb}", tag=f"pn{pb}")
                bsl = slice(b * F_PE, (b + 1) * F_PE)
                for sb in range(F_PE // 512):
                    psl = slice(sb * 512, (sb + 1) * 512)
                    ssl = slice(b * F_PE + sb * 512, b * F_PE + (sb + 1) * 512)
                    for j in range(3):
                        nc.tensor.matmul(out=pnum[:, psl], lhsT=dg[j],
                                         rhs=s_t[j][:, ssl],
                                         start=(j == 0), stop=(j == 2))
                # tt = num * recip1nr(den + eps)  (custom: SBUF in0, PSUM in1)
                nc.vector._custom_dve(
                    OP_RECIPMUL, out=tt[:, bsl],
                    in0=den[:, bsl], in1=pnum, s0=RA, s1=RB, imm2=EPS_W)
            # out0 = (xv - muv) * tt with accum -> osum  (DVE STT)
            nc.vector.scalar_tensor_tensor(
                out=out0[:, sl], in0=x_sb["v"][:, sl], scalar=muv, in1=tt,
                op0=Alu.subtract, op1=Alu.mult,
                accum_out=osum[:, h:h + 1])
            # osq via ScalarE Square+accum
            scr2 = work.tile([DL, F_MAIN], BF16, name="scr2", tag="stg")
            nc.scalar.activation(scr2, out0[:, sl], Act.Square,
                                 accum_out=osq[:, h:h + 1])

        # ---- phase 3: final norm constants --------------------------------
        sum_o = consts.tile([DL, 1], F32, name="sum_o", tag="sum_o")
        nc.vector.tensor_reduce(sum_o, osum, axis=mybir.AxisListType.X, op=Alu.add)
        sq_o = consts.tile([DL, 1], F32, name="sq_o", tag="sq_o")
        nc.vector.tensor_reduce(sq_o, osq, axis=mybir.AxisListType.X, op=Alu.add)
        mean_o = consts.tile([DL, 1], F32, name="mean_o", tag="mean_o")
        nc.vector.tensor_scalar_mul(out=mean_o, in0=sum_o, scalar1=1.0 / BT)
        msq_o = consts.tile([DL, 1], F32, name="msq_o", tag="msq_o")
        nc.vector.tensor_mul(msq_o, mean_o, mean_o)
        var_o = consts.tile([DL, 1], F32, name="var_o", tag="var_o")
        nc.vector.scalar_tensor_tensor(
            out=var_o, in0=sq_o, scalar=1.0 / BT, in1=msq_o,
            op0=Alu.mult, op1=Alu.subtract,
        )
        nc.vector.tensor_scalar_add(out=var_o, in0=var_o, scalar1=EPS_NORM)
        rs_o = _emit_rsqrt(nc, consts, var_o, 1, "o")
        fs = consts.tile([DL, 1], F32, name="fs", tag="fs")
        nc.vector.tensor_mul(fs, g_out_sb, rs_o)
        fbt = consts.tile([DL, 1], F32, name="fbt", tag="fbt")
        nc.vector.tensor_mul(fbt, mean_o, fs)
        fb = consts.tile([DL, 1], F32, name="fb", tag="fb")
        nc.vector.tensor_sub(fb, b_out_sb, fbt)

        # ---- phase 4: final affine (ScalarE) + store via swdge ------------
        # Output DMAs go on the GpSimd software-DGE queue so the two HW
        # queues carry only inputs; body n's stores no longer block body
        # n+1's loads.
        F_FIN = 2048
        for i in range(BT // F_FIN):
            sl = slice(i * F_FIN, (i + 1) * F_FIN)
            stg = work.tile([DL, F_FIN], BF16, name="stg", tag="stg")
            if i % 2 == 0:
                nc.vector.tensor_scalar(out=stg, in0=out0[:, sl], scalar1=fs,
                                        scalar2=fb, op0=Alu.mult, op1=Alu.add)
            else:
                nc.scalar.activation(stg, out0[:, sl], Act.Identity,
                                     bias=fb, scale=fs)
            nc.gpsimd.dma_start(out=dram["out"][:, sl], in_=stg)


def build_program(reps=1, variant=None):
    nc = bacc.Bacc("TRN2", num_devices=N_CORES)
    dram = {}
    for p in ("q", "k", "v"):
        dram["x" + p] = nc.dram_tensor("x" + p, [DL, BT], BF16, kind="ExternalInput").ap()
        dram["xt" + p] = nc.dram_tensor(
            "xt" + p, [DL, BT // DL, DL + 1], BF16, kind="ExternalInput").ap()
    dram["usq_all"] = nc.dram_tensor("usq_all", [DL, 9], F32, kind="ExternalInput").ap()
    dram["ug_all"] = nc.dram_tensor("ug_all", [DL, 9], F32, kind="ExternalInput").ap()
    dram["g_out"] = nc.dram_tensor("g_out", [DL, 1], F32, kind="ExternalInput").ap()
    dram["b_out"] = nc.dram_tensor("b_out", [DL, 1], F32, kind="ExternalInput").ap()
    dram["ident"] = nc.dram_tensor("ident", [DL, DL], BF16, kind="ExternalInput").ap()
    dram["out"] = nc.dram_tensor("out", [DL, BT], BF16, kind="ExternalOutput").ap()

    import contextlib
    with tile.TileContext(nc) as tc:
        with contextlib.ExitStack() as ctx:
            pools = (
                ctx.enter_context(tc.tile_pool(name="resident", bufs=2)),
                ctx.enter_context(tc.tile_pool(name="out0p", bufs=1)),
                ctx.enter_context(tc.tile_pool(name="consts", bufs=2)),
                ctx.enter_context(tc.tile_pool(name="work", bufs=2)),
                ctx.enter_context(tc.tile_pool(name="psum", bufs=1, space="PSUM")),
            )
            for _ in range(reps):
                _emit_body(nc, tc, dram, pools)
    nc.compile()
    return nc


def _softplus(x):
    return np.log1p(np.exp(-np.abs(x))) + np.maximum(x, 0.0)


def _host_params(w, b, a, g, beta):
    Q = np.linalg.qr(np.asarray(a, dtype=np.float64))[0].astype(np.float32)
    u = np.einsum("di,dij->dj", _softplus(np.asarray(w, np.float64)).astype(np.float32), Q)
    return u, u * np.asarray(g, np.float32)


def _reference_fallback(x, wq, bq, aq, gq, betaq, wk, bk, ak, gk, betak,
                        wv, bv, av, gv, betav, g_out, b_out):
    def block(xi, w, b, a, g, beta):
        h = xi[..., None] * _softplus(w) + b
        Q = np.linalg.qr(a)[0]
        h = np.einsum("btdi,dij->btdj", h, Q)
        mean = h.mean(axis=(0, 1))
        var = h.var(axis=(0, 1))
        return (h - mean) / np.sqrt(var + EPS_NORM) * g + beta

    d = D
    Qp = block(x[..., :d], wq, bq, aq, gq, betaq)
    Kp = block(x[..., d:2 * d], wk, bk, ak, gk, betak)
    Vp = block(x[..., 2 * d:], wv, bv, av, gv, betav)
    scores = 1.0 / (1.0 + np.exp(-GAMMA * (Qp * Kp)))
    weights = scores / (scores.sum(axis=-1, keepdims=True) + EPS_W)
    out = (weights * Vp).sum(axis=-1)
    mean = out.mean(axis=(0, 1))
    var = out.var(axis=(0, 1))
    return ((out - mean) / np.sqrt(var + EPS_NORM) * g_out + b_out).astype(np.float32)


_NC_CACHE = {}

VARIANT = "v2"


def _get_program(reps=1, variant=None):
    if variant is None:
        variant = VARIANT
    key = (reps, variant)
    if key not in _NC_CACHE:
        _NC_CACHE[key] = build_program(reps, variant)
    return _NC_CACHE[key]


def _make_in_maps(x, params):
    import ml_dtypes
    x2 = np.asarray(x, np.float32).reshape(BT, 3 * D)
    xt = np.ascontiguousarray(
        x2.reshape(BT, 3 * N_CORES, DL).transpose(1, 2, 0)).astype(ml_dtypes.bfloat16)
    # pack u/ug as [D, 9] (q|k|v); gamma folded into ug_q so cmat = Aq'*Ak
    u_all = np.concatenate([params[p][0] for p in ("q", "k", "v")], axis=1)
    ug_all = np.concatenate(
        [params["q"][1] * GAMMA, params["k"][1], params["v"][1]], axis=1)
    NB = BT // DL
    in_maps = []
    for c in range(N_CORES):
        m = {}
        for pi, p in enumerate(("q", "k", "v")):
            xc = xt[pi * N_CORES + c]          # [DL, BT] bf16
            m["x" + p] = xc
            # [f%128, block, 129] = [x_block^T | ones]
            xtr = np.ones((DL, NB, DL + 1), dtype=ml_dtypes.bfloat16)
            xtr[:, :, :DL] = xc.reshape(DL, NB, DL).transpose(2, 1, 0)
            m["xt" + p] = xtr
        m["usq_all"] = np.ascontiguousarray((u_all * u_all)[c * DL:(c + 1) * DL])
        m["ug_all"] = np.ascontiguousarray(ug_all[c * DL:(c + 1) * DL])
        m["g_out"] = np.ascontiguousarray(params["g_out"][c * DL:(c + 1) * DL, None])
        m["b_out"] = np.ascontiguousarray(params["b_out"][c * DL:(c + 1) * DL, None])
        m["ident"] = np.eye(DL, dtype=ml_dtypes.bfloat16)
        in_maps.append(m)
    return in_maps


def kernel(x, wq, bq, aq, gq, betaq, wk, bk, ak, gk, betak,
           wv, bv, av, gv, betav, g_out, b_out):
    if (np.any(np.asarray(betaq)) or np.any(np.asarray(betak))
            or np.any(np.asarray(betav))):
        return _reference_fallback(x, wq, bq, aq, gq, betaq, wk, bk, ak, gk,
                                   betak, wv, bv, av, gv, betav, g_out, b_out)

    params = {
        "q": _host_params(wq, bq, aq, gq, betaq),
        "k": _host_params(wk, bk, ak, gk, betak),
        "v": _host_params(wv, bv, av, gv, betav),
        "g_out": np.asarray(g_out, np.float32),
        "b_out": np.asarray(b_out, np.float32),
    }
    nc = _get_program()
    in_maps = _make_in_maps(x, params)
    try:
        per_core = _run_cached(nc, in_maps)
    except Exception:
        res = bass_utils.run_bass_kernel_spmd(
            nc, in_maps, core_ids=list(range(N_CORES)))
        per_core = [res.results[c]["out"] for c in range(N_CORES)]
    out = np.empty((BT, D), np.float32)
    for c in range(N_CORES):
        out[:, c * DL:(c + 1) * DL] = np.asarray(per_core[c], np.float32).T
    return out.reshape(B, T, D)


_RUNNER_CACHE = {}


def _run_cached(nc, in_maps):
    """Jit the bass_exec shard_map once; later calls only restage inputs."""
    key = id(nc)
    if key not in _RUNNER_CACHE:
        import jax
        from jax.sharding import Mesh, PartitionSpec, NamedSharding
        try:
            from jax import shard_map
        except ImportError:
            from jax.experimental.shard_map import shard_map
        from concourse import mybir as _mb
        from concourse.bass2jax import (
            _bass_exec_p, install_neuronx_cc_hook, partition_id_tensor)

        install_neuronx_cc_hook()
        pname = nc.partition_id_tensor.name if nc.partition_id_tensor else None
        in_names, out_names, out_avals, zero_outs = [], [], [], []
        for alloc in nc.m.functions[0].allocations:
            if not isinstance(alloc, _mb.MemoryLocationSet):
                continue
            name = alloc.memorylocations[0].name
            if alloc.kind == "ExternalInput":
                if name != pname:
                    in_names.append(name)
            elif alloc.kind == "ExternalOutput":
                out_names.append(name)
                shp = tuple(alloc.tensor_shape)
                dt_np = _mb.dt.np(alloc.dtype)
                out_avals.append(jax.core.ShapedArray(shp, dt_np))
                zero_outs.append(np.zeros(shp, dt_np))
        all_in = list(in_names) + list(out_names)
        if pname is not None:
            all_in.append(pname)

        def _body(*args):
            operands = list(args)
            if pname is not None:
                operands.append(partition_id_tensor())
            return tuple(_bass_exec_p.bind(
                *operands, out_avals=tuple(out_avals), in_names=tuple(all_in),
                out_names=tuple(out_names), lowering_input_output_aliases=(),
                sim_require_finite=True, sim_require_nnan=True, nc=nc))

        devices = jax.devices()[:N_CORES]
        mesh = Mesh(np.asarray(devices), ("core",))
        nspec = (PartitionSpec("core"),) * (len(in_names) + len(out_names))
        try:
            smapped = shard_map(_body, mesh=mesh, in_specs=nspec,
                                out_specs=(PartitionSpec("core"),) * len(out_names),
                                check_rep=False)
        except TypeError:
            smapped = shard_map(_body, mesh=mesh, in_specs=nspec,
                                out_specs=(PartitionSpec("core"),) * len(out_names),
                                check_vma=False)
        jitted = jax.jit(smapped, keep_unused=True)
        sh = NamedSharding(mesh, PartitionSpec("core"))
        zconcat = [
            jax.device_put(
                np.zeros((N_CORES * z.shape[0], *z.shape[1:]), z.dtype), sh)
            for z in zero_outs]
        _RUNNER_CACHE[key] = (jitted, in_names, out_names, out_avals, sh, zconcat)
    import jax
    jitted, in_names, out_names, out_avals, sh, zconcat = _RUNNER_CACHE[key]
    args = [
        jax.device_put(
            np.concatenate([in_maps[c][nm] for c in range(N_CORES)], axis=0), sh)
        for nm in in_names]
    outs = jitted(*args, *zconcat)
    oi = out_names.index("out")
    full = np.asarray(outs[oi]).reshape(N_CORES, *out_avals[oi].shape)
    return [full[c] for c in range(N_CORES)]
